# revision 20
# baseline (speedup 1.0000x reference)
"""Trainium2 Bass kernel for nn_CrossAttention (b=8, c=128, hw=4096, dim=64).

Sharding: data-parallel over batch — one batch element per NeuronCore (8 cores).

Per-core algorithm (channel-major [c, t] layout, t = h*w = 4096 tokens):
  - LayerNorm over channels is folded algebraically:
      G = W' @ x + (-colsum(W')) (x) mu     (rank-1 mean-subtract fused into
                                             the PE accumulation, K=1 matmul)
      proj = relu(G * r_bcast + b')
    where W' = W * ln_w and b' = W @ ln_b + b are host-folded, mu/r are the
    per-token channel stats, r = rsqrt(var+eps) = exp(-0.5*ln(var+eps))
    (keeps ACT on the single exp+ln table set).
  - Channel-dim stats via ones-vector matmuls on PE, col-tiled so the four
    stat rows (sum x, sum x^2, sum c, sum c^2) land on psum partitions
    0/32/64/96 of one bank; lane-parallel stat math on [128, 32] reshapes
    (layout: element (p, m) = token 32p + m).
  - Attention with transposed scores: sT[tj, ti] = k_blk.T @ q so softmax's
    exp applies per strip and pT feeds the A@V matmul with no transposes.
    No max-subtraction (scores are bounded); a constant shift cancels in the
    normalization. Softmax denominator via a fused ones-column in V (M=65).
  - Division by l deferred through the output projection (it commutes); bout
    enters as an extra K-row of the output matmul scaled by l, so the final
    normalize is a single tensor_tensor multiply.
All matmuls use float32r (full-rate fp32 on the PE at N=512).
"""

import sys

if "/opt/trn_rl_repo" not in sys.path:
    sys.path.insert(0, "/opt/trn_rl_repo")

import numpy as np

B = 8
C = 128  # channels (x_dim == ctx_dim)
D = 64  # attention dim
T = 4096  # tokens = 64*64
EPS = 1e-5
SCALE = float(D) ** -0.5
SHIFT = 2.0  # constant subtracted inside exp; cancels in softmax normalization

_CACHE = {}


def _build_program():
    import contextlib

    import concourse.bass as bass
    import concourse.bacc as bacc
    import concourse.mybir as mybir
    import concourse.tile as tile

    f32 = mybir.dt.float32
    f32r = mybir.dt.float32r
    FT = mybir.ActivationFunctionType
    OP = mybir.AluOpType

    nc = bacc.Bacc("TRN2", target_bir_lowering=False, debug=False, num_devices=B)

    x_d = nc.dram_tensor("x", [C, T], f32r, kind="ExternalInput")
    c_d = nc.dram_tensor("ctx", [C, T], f32r, kind="ExternalInput")
    wq_d = nc.dram_tensor("wq", [C, D], f32r, kind="ExternalInput")  # (Wq*ln_w).T
    wkv_d = nc.dram_tensor("wkv", [C, 2 * D], f32r, kind="ExternalInput")
    sq_d = nc.dram_tensor("sq", [1, D], f32r, kind="ExternalInput")  # -colsum
    skv_d = nc.dram_tensor("skv", [1, 2 * D], f32r, kind="ExternalInput")
    bq_d = nc.dram_tensor("bq", [D, 1], f32, kind="ExternalInput")
    bkv_d = nc.dram_tensor("bkv", [2 * D, 1], f32, kind="ExternalInput")
    wo_d = nc.dram_tensor("wo", [D + 1, C], f32r, kind="ExternalInput")  # [Wout.T; bout]
    id_d = nc.dram_tensor("ident", [D, D], f32, kind="ExternalInput")
    out_d = nc.dram_tensor("out", [C, T], f32, kind="ExternalOutput")
    rx_scr = nc.dram_tensor("rx_scr", [T], f32r)
    rc_scr = nc.dram_tensor("rc_scr", [T], f32r)
    rl_scr = nc.dram_tensor("rl_scr", [T], f32r)


    NJ = T // 128  # 32 key strips
    NPASS = 2
    SPAN = T // NPASS  # 2048 ti per pass
    NS = T // 128  # 32 cols in the [128, NS] stat reshape

    with (
        tile.TileContext(nc) as tc,
        nc.allow_low_precision(
            reason="float32r tensors feed full-rate PE matmuls; values are "
            "fp32-resident and only rounded inside the PE"
        ),
    ):
        with contextlib.ExitStack() as ctx:
            const = ctx.enter_context(tc.tile_pool(name="const", bufs=1))
            big = ctx.enter_context(tc.tile_pool(name="big", bufs=1))
            st32 = ctx.enter_context(tc.tile_pool(name="st32", bufs=1))
            sqp = ctx.enter_context(tc.tile_pool(name="sqp", bufs=4))
            prep = ctx.enter_context(tc.tile_pool(name="prep", bufs=2))
            bcp = ctx.enter_context(tc.tile_pool(name="bcp", bufs=2))
            stgp = ctx.enter_context(tc.tile_pool(name="stgp", bufs=2))
            strow = ctx.enter_context(tc.tile_pool(name="strow", bufs=4))
            ptp = ctx.enter_context(tc.tile_pool(name="ptp", bufs=4))
            outp = ctx.enter_context(tc.tile_pool(name="outp", bufs=2))

            # ---- constants ----
            wq_sb = const.tile([C, D], f32r)
            wkv_sb = const.tile([C, 2 * D], f32r)
            sq_sb = const.tile([1, D], f32r)
            skv_sb = const.tile([1, 2 * D], f32r)
            bq_sb = const.tile([D, 1], f32)
            bkv_sb = const.tile([2 * D, 1], f32)
            wo_sb = const.tile([D + 1, C], f32r)
            id_sb = const.tile([C, D], f32)
            ones_sb = const.tile([C, 32], f32r)
            eps_sb = const.tile([C, 1], f32)
            shift_sb = const.tile([C, 1], f32)
            nc.sync.dma_start(wq_sb[:], wq_d.ap())
            nc.sync.dma_start(wkv_sb[:], wkv_d.ap())
            nc.sync.dma_start(sq_sb[:], sq_d.ap())
            nc.sync.dma_start(skv_sb[:], skv_d.ap())
            nc.sync.dma_start(bq_sb[:], bq_d.ap())
            nc.sync.dma_start(bkv_sb[:], bkv_d.ap())
            nc.sync.dma_start(wo_sb[:], wo_d.ap())
            # identity needed at partitions 64..127 (v lives there in kv_sb)
            nc.sync.dma_start(id_sb[D : 2 * D, :], id_d.ap())
            nc.vector.memset(ones_sb[:].bitcast(f32), 1.0)
            nc.vector.memset(eps_sb[:], EPS)
            nc.vector.memset(shift_sb[:], -SHIFT)

            # ---- big persistent tensors ----
            x_sb = big.tile([C, T], f32r)
            c_sb = big.tile([C, T], f32r)
            q_sb = big.tile([D, T], f32r)
            kv_sb = big.tile([2 * D, T], f32r)
            v_tok = big.tile([128, NJ, D + 1], f32r)
            attn_sb = big.tile([D + 1, T], f32r)

            for n in range(4):
                sl = slice(n * 1024, (n + 1) * 1024)
                nc.sync.dma_start(x_sb[:, sl], x_d.ap()[:, sl])
                nc.sync.dma_start(c_sb[:, sl], c_d.ap()[:, sl])

            # v' ones column: preset whole v_tok to 1.0; transposes fill cols 0:D
            nc.vector.memset(v_tok[:].bitcast(f32), 1.0)

            # ---- phase A1: channel stats ----
            with tc.tile_pool(name="pst", bufs=4, space="PSUM") as pstp:
                xs_t = st32.tile([128, NS], f32r)
                xss_t = st32.tile([128, NS], f32r)
                cs_t = st32.tile([128, NS], f32r)
                css_t = st32.tile([128, NS], f32r)
                for n in range(8):
                    sl = slice(n * 512, (n + 1) * 512)
                    c4 = slice(n * 4, (n + 1) * 4)
                    xsq = sqp.tile([C, 512], f32r, tag="sq")
                    csq = sqp.tile([C, 512], f32r, tag="sq")
                    nc.vector.tensor_mul(xsq[:], x_sb[:, sl], x_sb[:, sl])
                    nc.vector.tensor_mul(csq[:], c_sb[:, sl], c_sb[:, sl])
                    for rhs, dst_t in (
                        (x_sb[:, sl], xs_t),
                        (xsq[:], xss_t),
                        (c_sb[:, sl], cs_t),
                        (csq[:], css_t),
                    ):
                        pst = pstp.tile([32, 512], f32, tag="pst")
                        nc.tensor.matmul(pst[:], ones_sb[:], rhs)
                        row = strow.tile([1, 512], f32r, tag="strow")
                        nc.vector.tensor_copy(row[:], pst[0:1, :])
                        # [1, 512] row -> [128, 4]: token 512n + 4p + i
                        nc.sync.dma_start(dst_t[:, c4], row[:])

                def stats_math(s_t, ss_t, pfx):
                    mu_t = st32.tile([128, NS], f32r, tag=pfx + "mu")
                    mu2_t = st32.tile([128, NS], f32r, tag=pfx + "mu2")
                    var_t = st32.tile([128, NS], f32r, tag=pfx + "var")
                    r_t = st32.tile([128, NS], f32r, tag=pfx + "r")
                    nc.vector.tensor_scalar_mul(mu_t[:], s_t[:], 1.0 / C)
                    nc.vector.tensor_mul(mu2_t[:], mu_t[:], mu_t[:])
                    nc.vector.scalar_tensor_tensor(
                        var_t[:], ss_t[:], 1.0 / C, mu2_t[:], OP.mult, OP.subtract
                    )
                    nc.scalar.activation(var_t[:], var_t[:], FT.Ln, bias=eps_sb[:])
                    nc.scalar.activation(r_t[:], var_t[:], FT.Exp, scale=-0.5)
                    return mu_t, r_t

                mux_t, rx_t = stats_math(xs_t, xss_t, "x")
                muc_t, rc_t = stats_math(cs_t, css_t, "c")
                scr_ap = lambda h: h.ap().rearrange(
                    "(c p i) -> p c i", c=8, p=128, i=4
                )
                nc.sync.dma_start(scr_ap(rx_scr), rx_t[:])
                nc.sync.dma_start(scr_ap(rc_scr), rc_t[:])

            # ---- phase A2: projections + v transpose ----
            with (
                tc.tile_pool(name="ppr", bufs=2, space="PSUM") as pprp,
                tc.tile_pool(name="ptr", bufs=2, space="PSUM") as ptrp,
            ):
                # projections: G = W' @ x - s (x) mu ; out = relu(G*r + b)
                def project(w_sb, s_sb, b_sb, src_sb, mu_t, r_scr, dst_sb, m):
                    for n in range(4):
                        mu_stg = stgp.tile([1, 1024], f32r, tag="mustg")
                        for m2 in range(2):
                            nc.sync.dma_start(
                                mu_stg[0:1, m2 * 512 : (m2 + 1) * 512],
                                mu_t[:, 8 * n + 4 * m2 : 8 * n + 4 * m2 + 4],
                            )
                        rbc = bcp.tile([128, 1024], f32r, tag="rbc")
                        nc.sync.dma_start(
                            rbc[0:m, :],
                            bass.AP(r_scr, n * 1024, [[0, m], [1, 1024]]),
                        )
                        ps = pprp.tile([128, 1024], f32, tag="pp")
                        for g in range(2):
                            sl = slice(n * 1024 + g * 512, n * 1024 + (g + 1) * 512)
                            po = ps[0:m, g * 512 : (g + 1) * 512]
                            nc.tensor.matmul(
                                po,
                                w_sb[:],
                                src_sb[:, sl],
                                start=True,
                                stop=False,
                            )
                            nc.tensor.matmul(
                                po,
                                s_sb[:],
                                mu_stg[:, g * 512 : (g + 1) * 512],
                                start=False,
                                stop=True,
                            )
                        sl4 = slice(n * 1024, (n + 1) * 1024)
                        pre = prep.tile([128, 1024], f32, tag="pre")
                        nc.vector.tensor_mul(pre[0:m, :], ps[0:m, :], rbc[0:m, :])
                        nc.vector.tensor_scalar(
                            dst_sb[:, sl4],
                            pre[0:m, :],
                            b_sb[:],
                            0.0,
                            op0=OP.add,
                            op1=OP.max,
                        )

                project(wq_sb, sq_sb, bq_sb, x_sb, mux_t, rx_scr, q_sb, D)
                project(wkv_sb, skv_sb, bkv_sb, c_sb, muc_t, rc_scr, kv_sb, 2 * D)

                # v (kv rows D..2D) -> token-major tiles [tj, d]
                for j in range(NJ):
                    tp = ptrp.tile([128, D], f32)
                    nc.tensor.matmul(
                        tp[:],
                        kv_sb[D : 2 * D, j * 128 : (j + 1) * 128].bitcast(f32),
                        id_sb[D : 2 * D, :],
                        is_transpose=True,
                    )
                    nc.vector.tensor_copy(v_tok[:, j, 0:D], tp[:])

            # ---- phase B: attention ----
            with (
                tc.tile_pool(name="pss", bufs=2, space="PSUM") as pssp,
                tc.tile_pool(name="pav", bufs=1, space="PSUM") as pavp,
            ):
                for p2 in range(NPASS):
                    pav = pavp.tile([D + 1, SPAN], f32)
                    for j in range(NJ):
                        kblk = kv_sb[0:D, j * 128 : (j + 1) * 128]
                        vblk = v_tok[:, j, :]
                        for h in range(2):
                            pss = pssp.tile([128, 1024], f32)
                            for g in range(2):
                                ti0 = p2 * SPAN + h * 1024 + g * 512
                                nc.tensor.matmul(
                                    pss[:, g * 512 : (g + 1) * 512],
                                    kblk,
                                    q_sb[:, ti0 : ti0 + 512],
                                )
                            pt = ptp.tile([128, 1024], f32r, tag="pt")
                            nc.scalar.activation(
                                pt[:], pss[:], FT.Exp, bias=shift_sb[:], scale=SCALE
                            )
                            for g in range(2):
                                co = h * 1024 + g * 512
                                nc.tensor.matmul(
                                    pav[:, co : co + 512],
                                    vblk,
                                    pt[:, g * 512 : (g + 1) * 512],
                                    start=(j == 0),
                                    stop=(j == NJ - 1),
                                )
                    nc.vector.tensor_copy(
                        attn_sb[:, p2 * SPAN : (p2 + 1) * SPAN], pav[:]
                    )

            # ---- phase C: 1/l and output projection ----
            with tc.tile_pool(name="pout", bufs=2, space="PSUM") as poutp:
                l_t = st32.tile([128, NS], f32r, tag="lt")
                rl_t = st32.tile([128, NS], f32r, tag="rlt")
                for n in range(8):
                    nc.sync.dma_start(
                        l_t[:, n * 4 : (n + 1) * 4],
                        attn_sb[D : D + 1, n * 512 : (n + 1) * 512],
                    )
                nc.vector.reciprocal(rl_t[:], l_t[:])
                nc.sync.dma_start(
                    rl_scr.ap().rearrange("(c p i) -> p c i", c=8, p=128, i=4),
                    rl_t[:],
                )

                for n in range(4):
                    rlbc = bcp.tile([128, 1024], f32r, tag="rbc")
                    nc.sync.dma_start(
                        rlbc[:], bass.AP(rl_scr, n * 1024, [[0, C], [1, 1024]])
                    )
                    po = poutp.tile([C, 1024], f32)
                    for g in range(2):
                        sl = slice(n * 1024 + g * 512, n * 1024 + (g + 1) * 512)
                        nc.tensor.matmul(
                            po[:, g * 512 : (g + 1) * 512],
                            wo_sb[:],
                            attn_sb[:, sl],
                        )
                    sl4 = slice(n * 1024, (n + 1) * 1024)
                    ot = outp.tile([C, 1024], f32)
                    nc.vector.tensor_mul(ot[:], po[:], rlbc[:])
                    nc.sync.dma_start(out_d.ap()[:, sl4], ot[:])

    nc.compile()
    return nc


def _get_program():
    if "nc" not in _CACHE:
        _CACHE["nc"] = _build_program()
    return _CACHE["nc"]


def _fold_weights(ln_x_w, ln_x_b, ln_c_w, ln_c_b, Wq, bq, Wkv, bkv, Wout, bout):
    f = np.float64
    Wq = np.asarray(Wq, f)
    Wkv = np.asarray(Wkv, f)
    Wout = np.asarray(Wout, f)
    wq_p = Wq * np.asarray(ln_x_w, f)[None, :]  # [D, C]
    wkv_p = Wkv * np.asarray(ln_c_w, f)[None, :]  # [2D, C]
    bq_p = Wq @ np.asarray(ln_x_b, f) + np.asarray(bq, f)
    bkv_p = Wkv @ np.asarray(ln_c_b, f) + np.asarray(bkv, f)
    wo_aug = np.concatenate([Wout.T, np.asarray(bout, f)[None, :]], axis=0)  # [D+1, C]
    return {
        "wq": np.ascontiguousarray(wq_p.T, np.float32),
        "wkv": np.ascontiguousarray(wkv_p.T, np.float32),
        "sq": np.ascontiguousarray(-wq_p.sum(axis=1)[None, :], np.float32),
        "skv": np.ascontiguousarray(-wkv_p.sum(axis=1)[None, :], np.float32),
        "bq": np.ascontiguousarray(bq_p[:, None], np.float32),
        "bkv": np.ascontiguousarray(bkv_p[:, None], np.float32),
        "wo": np.ascontiguousarray(wo_aug, np.float32),
        "ident": np.eye(D, dtype=np.float32),
    }


def _run(inputs, trace=False):
    from concourse.bass_utils import run_bass_kernel_spmd

    nc = _get_program()
    x = np.asarray(inputs["x"], np.float32)
    ctx = np.asarray(inputs["context"], np.float32)
    w = _fold_weights(
        inputs["ln_x_w"], inputs["ln_x_b"], inputs["ln_c_w"], inputs["ln_c_b"],
        inputs["Wq"], inputs["bq"], inputs["Wkv"], inputs["bkv"],
        inputs["Wout"], inputs["bout"],
    )
    in_maps = []
    for i in range(B):
        m = dict(w)
        m["x"] = np.ascontiguousarray(x[i].reshape(C, T))
        m["ctx"] = np.ascontiguousarray(ctx[i].reshape(C, T))
        in_maps.append(m)
    res = run_bass_kernel_spmd(nc, in_maps, list(range(B)), trace=trace)
    h = int(np.sqrt(T))
    out = np.stack([res.results[i]["out"].reshape(C, h, h) for i in range(B)])
    return out, res


def kernel(**inputs) -> np.ndarray:
    out, _ = _run(inputs, trace=False)
    return out


def bench(inputs):
    out, res = _run(inputs, trace=True)
    return out, res.exec_time_ns
